# revision 17
# baseline (speedup 1.0000x reference)
"""Trainium2 Bass kernel for the attention+GRU decoder (nn_Decoder).

Shapes: B=128, T=48, I=128, H=512.  8 NeuronCores, data-parallel over the
batch (16 batch elements per core); the whole 48-step recurrence runs
on-device per core with no cross-core communication.

Math (per batch element b, per step t, h = h_{t-1}):
  w1   = h @ W1.T + b1                       [H]
  e_t  = tanh(w1 + w2_bt) @ W3 + b3          [T]    (w2 precomputed, b3 drops in softmax)
  a    = softmax(e)                          [T]
  c    = sum_t a_t * dx_bt                   [I]
  gi   = [c, x_t] @ (W_ih @ W4).T + (W_ih@b4 + b_ih)   (W4 folded into W_ih => y0 eliminated)
  gh   = h @ W_hh.T + b_hh
  r,z  = sigmoid(...); n = tanh(in + r*hn); h' = (1-z)n + z h
  sigmoid(x) = 0.5 + 0.5*tanh(x/2)  (keeps ACT in the exp_and_others table set)

Device layouts are feature-major ("vertical"): partitions = feature dim,
free dim = batch (16) or (batch, tau).  All weight packing / transposes /
bias folds happen on the host in numpy.
"""

import numpy as np

B, T, I, H = 128, 48, 128, 512
NCORES = 8
BL = B // NCORES  # 16 batch elements per core
BT = BL * T       # 768
G = 3 * H         # 1536 gate dim
NKH = H // 128    # 4   h'-tiles
NMG = G // 128    # 12  gate m-tiles

_PROGRAM_CACHE = {}
DEBUG_DUMP = False
N_STEPS = T  # recurrence steps (debug knob; attention window stays T)


def _build_program():
    """Build the raw-bass program (one NeuronCore, SPMD across 8)."""
    import concourse.bass as bass
    import concourse.mybir as mybir
    from contextlib import ExitStack

    f32 = mybir.dt.float32
    Tanh = mybir.ActivationFunctionType.Tanh
    Exp = mybir.ActivationFunctionType.Exp
    Copy = mybir.ActivationFunctionType.Copy
    add = mybir.AluOpType.add
    sub = mybir.AluOpType.subtract
    mult = mybir.AluOpType.mult
    AX = mybir.AxisListType.X

    # Raw-bass kernel: same-engine ordering is guaranteed by the in-order
    # engines; the sim's race detector doesn't model that, so disable it.
    nc = bass.Bass(detect_race_conditions=False)

    # ---- DRAM I/O (per-core tensors) ----
    d_dxT = nc.dram_tensor("dxT", [128, BT], f32, kind="ExternalInput")
    d_w2T = nc.dram_tensor("w2T", [128, NKH * BT], f32, kind="ExternalInput")
    d_gixT = nc.dram_tensor("gixT", [128, NMG * T * BL], f32, kind="ExternalInput")
    d_hT0 = nc.dram_tensor("hT0", [128, NKH * BL], f32, kind="ExternalInput")
    d_W1T = nc.dram_tensor("W1T", [128, NKH * NKH * 128], f32, kind="ExternalInput")
    d_WhhT = nc.dram_tensor("WhhT", [128, NKH * NMG * 128], f32, kind="ExternalInput")
    d_WacT = nc.dram_tensor("WacT", [128, NMG * 128], f32, kind="ExternalInput")
    d_W3r = nc.dram_tensor("W3r", [128, NKH * 128], f32, kind="ExternalInput")
    d_Wfc = nc.dram_tensor("Wfcc", [128, NKH], f32, kind="ExternalInput")

    d_re = nc.dram_tensor("out_re", [1, BL], f32, kind="ExternalOutput")
    d_dbg = {}
    if DEBUG_DUMP:
        for nm, w in [("cn", BL), ("cu", BL), ("sum", BL), ("rcp", BL),
                      ("adx", BT), ("arep", BT),
                      ("prerz", 8 * BL), ("trz", 8 * BL),
                      ("inn", NKH * BL), ("trhn", NKH * BL), ("q", NKH * BL),
                      ("npre", NKH * BL), ("tn", NKH * BL), ("gh", NMG * BL),
                      ("gi", NMG * BL), ("w1", NKH * BL)]:
            d_dbg[nm] = nc.dram_tensor("dbg_" + nm, [128, w], f32,
                                       kind="ExternalOutput")
    d_outh = nc.dram_tensor("out_h", [128, NKH * BL], f32, kind="ExternalOutput")
    d_aun = nc.dram_tensor("out_aun", [1, BT], f32, kind="ExternalOutput")
    d_asum = nc.dram_tensor("out_asum", [1, BL], f32, kind="ExternalOutput")

    ctx = ExitStack()
    _n = [0]

    def sb(shape):
        _n[0] += 1
        return ctx.enter_context(nc.sbuf_tensor(f"sbt{_n[0]}", shape, f32))

    def ps(shape):
        _n[0] += 1
        return ctx.enter_context(nc.psum_tensor(f"pst{_n[0]}", shape, f32))

    with ctx:
        # ---- SBUF ----
        sb_dxT = sb([128, BT])
        sb_w2T = sb([128, NKH * BT])
        sb_gixT = sb([128, NMG * T * BL])
        sb_hT = sb([128, NKH * BL])          # recurrent state hT
        sb_W1T = sb([128, NKH * NKH * 128])
        sb_WhhT = sb([128, NKH * NMG * 128])
        sb_WacT = sb([128, NMG * 128])
        sb_W3r = sb([128, NKH * 128])
        sb_Wfc = sb([128, NKH])

        sb_z = sb([128, NKH * BT])           # w1 (+) w2, then its tanh
        sb_tanh = sb([128, NKH * BT])
        sb_arep = sb([128, BT])              # exp(e) un-normalized, replicated
        sb_adx = sb([128, BT])               # a * dxT
        sb_cu = sb([128, BL])                # unnormalized context (vertical)
        sb_sum = sb([128, BL])               # sum_tau a (replicated rows)
        sb_rcp = sb([128, BL])
        sb_cn = sb([128, BL])                # normalized context
        sb_prerz = sb([128, 8 * BL])
        sb_tmprz = sb([128, 8 * BL])
        sb_trz = sb([128, 8 * BL])
        sb_inn = sb([128, NKH * BL])
        sb_trhn = sb([128, NKH * BL])
        sb_q = sb([128, NKH * BL])
        sb_npre = sb([128, NKH * BL])
        sb_tn = sb([128, NKH * BL])
        sb_p = sb([128, NKH * BL])
        sb_m = sb([128, NKH * BL])
        sb_d2 = sb([128, NKH * BL])
        sb_u = sb([128, NKH * BL])
        sb_re = sb([1, BL])
        sb_dbg_gh = sb([128, NMG * BL]) if DEBUG_DUMP else None
        sb_dbg_gi = sb([128, NMG * BL]) if DEBUG_DUMP else None
        sb_dbg_w1 = sb([128, NKH * BL]) if DEBUG_DUMP else None

        # ---- PSUM (8 banks total) ----
        ps_w1 = ps([128, NKH * BL])          # w1 vertical [h'-tile, b]
        ps_g = ps([128, NMG * BL])           # gh (all 12 m-tiles)
        ps_gi = ps([128, NMG * BL])          # gi (all 12 m-tiles)
        ps_e = ps([128, BT])                 # energies, replicated over partitions
        ps_fc = ps([1, BL])

        s_pe = ctx.enter_context(nc.semaphore("s_pe"))
        s_act = ctx.enter_context(nc.semaphore("s_act"))
        s_dve = ctx.enter_context(nc.semaphore("s_dve"))
        s_dma = ctx.enter_context(nc.semaphore("s_dma"))
        s_out = ctx.enter_context(nc.semaphore("s_out"))
        s_f = ctx.enter_context(nc.semaphore("s_f"))

        block = ctx.enter_context(nc.Block())

        n_in_dmas = 9

        @block.gpsimd
        def _(eng):
            for dst, src in [
                (sb_hT, d_hT0), (sb_W1T, d_W1T), (sb_WhhT, d_WhhT),
                (sb_WacT, d_WacT), (sb_W3r, d_W3r), (sb_Wfc, d_Wfc),
                (sb_dxT, d_dxT), (sb_w2T, d_w2T), (sb_gixT, d_gixT),
            ]:
                eng.dma_start(dst[:, :], src[:, :]).then_inc(s_dma, 16)

        # ------------- PE -------------
        @block.tensor
        def _(eng):
            mm = nc.tensor.matmul
            eng.wait_ge(s_dma, 16 * n_in_dmas)
            for t in range(N_STEPS):
                # P1: w1 = h @ W1.T   (vertical out [h', b])
                if t > 0:
                    eng.wait_ge(s_dve, 5 * (t - 1) + 5)
                last = None
                for mj in range(NKH):
                    for kj in range(NKH):
                        last = mm(ps_w1[:, mj * BL:(mj + 1) * BL],
                                  sb_W1T[:, (kj * NKH + mj) * 128:(kj * NKH + mj + 1) * 128],
                                  sb_hT[:, kj * BL:(kj + 1) * BL],
                                  start=(kj == 0), stop=(kj == NKH - 1))
                last.then_inc(s_pe, 1)  # 3t+1

                # P2: gh = W_hh @ h  -> ps_g (all 12 m-tiles)
                for mj in range(NMG):
                    for kj in range(NKH):
                        mm(ps_g[:, mj * BL:(mj + 1) * BL],
                           sb_WhhT[:, (kj * NMG + mj) * 128:(kj * NMG + mj + 1) * 128],
                           sb_hT[:, kj * BL:(kj + 1) * BL],
                           start=(kj == 0), stop=(kj == NKH - 1))

                # P3: e = W3 . tanh(z), output replicated over all partitions
                eng.wait_ge(s_act, 4 * t + 1)
                for kj in range(NKH):
                    mm(ps_e[:, 0:512], sb_W3r[:, kj * 128:(kj + 1) * 128],
                       sb_tanh[:, kj * BT:kj * BT + 512],
                       start=(kj == 0), stop=(kj == NKH - 1))
                last = None
                for kj in range(NKH):
                    last = mm(ps_e[:, 512:BT], sb_W3r[:, kj * 128:(kj + 1) * 128],
                              sb_tanh[:, kj * BT + 512:(kj + 1) * BT],
                              start=(kj == 0), stop=(kj == NKH - 1))
                last.then_inc(s_pe, 1)  # 3t+2

                # P5: gi = Wa_c @ cT (all 12 m-tiles, k = one 128 tile)
                eng.wait_ge(s_dve, 5 * t + 2)
                last = None
                for mj in range(NMG):
                    last = mm(ps_gi[:, mj * BL:(mj + 1) * BL],
                              sb_WacT[:, mj * 128:(mj + 1) * 128], sb_cn[:, :],
                              start=True, stop=True)
                last.then_inc(s_pe, 1)  # 3t+3

            # final: re = Wfc . h_final
            eng.wait_ge(s_dve, 5 * (N_STEPS - 1) + 5)
            last = None
            for kj in range(NKH):
                last = mm(ps_fc[0:1, :], sb_Wfc[:, kj:kj + 1],
                          sb_hT[:, kj * BL:(kj + 1) * BL],
                          start=(kj == 0), stop=(kj == NKH - 1), skip_group_check=True)
            last.then_inc(s_pe, 1)  # 4T+1

        # ------------- ACT -------------
        @block.scalar
        def _(eng):
            act = nc.scalar.activation
            for t in range(N_STEPS):
                eng.wait_ge(s_dve, 5 * t + 1)
                act(sb_tanh[:, :], sb_z[:, :], Tanh).then_inc(s_act, 1)  # 4t+1
                eng.wait_ge(s_pe, 3 * t + 2)
                act(sb_arep[:, :], ps_e[:, :], Exp).then_inc(s_act, 1)  # 4t+2
                eng.wait_ge(s_dve, 5 * t + 3)
                act(sb_trz[:, :], sb_prerz[:, :], Tanh, scale=0.5).then_inc(s_act, 1)  # 4t+3
                eng.wait_ge(s_dve, 5 * t + 4)
                act(sb_tn[:, :], sb_npre[:, :], Tanh).then_inc(s_act, 1)  # 4t+4
            # evac final fc psum
            eng.wait_ge(s_pe, 3 * N_STEPS + 1)
            act(sb_re[0:1, :], ps_fc[0:1, :], Copy).then_inc(s_act, 1)

        # ------------- DVE -------------
        @block.vector
        def _(eng):
            v = nc.vector
            z_out = sb_z[:, :].rearrange("p (j b t) -> p j b t", j=NKH, b=BL)
            w2_in = sb_w2T[:, :].rearrange("p (j b t) -> p j b t", j=NKH, b=BL)
            w1_in = ps_w1[:, :].rearrange("p (j b) -> p j b", j=NKH) \
                               .unsqueeze(3).broadcast_to([128, NKH, BL, T])
            gix_rz = sb_gixT[:, :].rearrange("p (m t b) -> p m t b", m=NMG, t=T)
            adx_v = sb_adx[:, :].rearrange("p (b t) -> p b t", b=BL)
            ar_v = sb_arep[:, :].rearrange("p (b t) -> p b t", b=BL)

            for t in range(N_STEPS):
                # z = w2T + w1 (broadcast over tau)
                eng.wait_ge(s_pe, 3 * t + 1)
                v.tensor_tensor(z_out, w2_in, w1_in, add).then_inc(s_dve, 1)  # 5t+1

                # context: a*dx, reductions, normalize
                eng.wait_ge(s_act, 4 * t + 2)
                v.tensor_tensor(sb_adx[:, :], sb_dxT[:, :], sb_arep[:, :], mult)
                v.tensor_reduce(sb_sum[:, :], ar_v, AX, add)
                v.tensor_reduce(sb_cu[:, :], adx_v, AX, add).then_inc(s_f, 1)
                # reciprocal races adjacent DVE ops on HW: fence before + after
                eng.wait_ge(s_f, 2 * t + 1)
                v.reciprocal(sb_rcp[:, :], sb_sum[:, :]).then_inc(s_f, 1)
                eng.wait_ge(s_f, 2 * t + 2)
                v.tensor_tensor(sb_cn[:, :], sb_cu[:, :], sb_rcp[:, :], mult) \
                 .then_inc(s_dve, 1)  # 5t+2

                # gates
                eng.wait_ge(s_pe, 3 * t + 3)
                v.tensor_tensor(
                    sb_tmprz[:, :].rearrange("p (m b) -> p m b", m=8),
                    ps_g[:, 0:8 * BL].rearrange("p (m b) -> p m b", m=8),
                    gix_rz[:, 0:8, t, :], add)
                v.tensor_tensor(sb_prerz[:, :], sb_tmprz[:, :],
                                ps_gi[:, 0:8 * BL], add).then_inc(s_dve, 1)  # 5t+3
                v.tensor_tensor(
                    sb_inn[:, :].rearrange("p (m b) -> p m b", m=NKH),
                    ps_gi[:, 8 * BL:NMG * BL].rearrange("p (m b) -> p m b", m=NKH),
                    gix_rz[:, 8:12, t, :], add)
                eng.wait_ge(s_act, 4 * t + 3)
                v.tensor_tensor(sb_trhn[:, :], sb_trz[:, 0:NKH * BL],
                                ps_g[:, 8 * BL:NMG * BL], mult)
                v.scalar_tensor_tensor(sb_q[:, :], ps_g[:, 8 * BL:NMG * BL], 0.5,
                                       sb_inn[:, :], mult, add)
                v.scalar_tensor_tensor(sb_npre[:, :], sb_trhn[:, :], 0.5,
                                       sb_q[:, :], mult, add).then_inc(s_dve, 1)  # 5t+4
                eng.wait_ge(s_act, 4 * t + 4)
                v.tensor_scalar_mul(sb_p[:, :], sb_hT[:, :], 0.5)
                v.scalar_tensor_tensor(sb_m[:, :], sb_tn[:, :], 0.5,
                                       sb_p[:, :], mult, add)
                v.scalar_tensor_tensor(sb_d2[:, :], sb_tn[:, :], 0.5,
                                       sb_p[:, :], mult, sub)
                v.tensor_tensor(sb_u[:, :], sb_trz[:, NKH * BL:8 * BL], sb_d2[:, :], mult)
                if DEBUG_DUMP and t == N_STEPS - 1:
                    v.tensor_copy(sb_dbg_gh[:, :], ps_g[:, :])
                    v.tensor_copy(sb_dbg_gi[:, :], ps_gi[:, :])
                    v.tensor_copy(sb_dbg_w1[:, :], ps_w1[:, :])
                v.tensor_tensor(sb_hT[:, :], sb_m[:, :], sb_u[:, :], sub) \
                 .then_inc(s_dve, 1)  # 5t+5

        # ------------- SYNC: outputs -------------
        @block.sync
        def _(eng):
            eng.wait_ge(s_act, 4 * N_STEPS + 1)  # fc evac done
            eng.dma_start(d_re[:, :], sb_re[0:1, :]).then_inc(s_out, 16)
            eng.wait_ge(s_dve, 5 * N_STEPS)
            eng.dma_start(d_aun[:, :], sb_arep[0:1, :]).then_inc(s_out, 16)
            eng.dma_start(d_asum[:, :], sb_sum[0:1, :]).then_inc(s_out, 16)
            eng.dma_start(d_outh[:, :], sb_hT[:, :]).then_inc(s_out, 16)
            n_out = 4
            if DEBUG_DUMP:
                for nm, sbuf in [("cn", sb_cn), ("cu", sb_cu), ("sum", sb_sum),
                                 ("rcp", sb_rcp), ("adx", sb_adx), ("arep", sb_arep),
                                 ("prerz", sb_prerz),
                                 ("trz", sb_trz), ("inn", sb_inn),
                                 ("trhn", sb_trhn), ("q", sb_q),
                                 ("npre", sb_npre), ("tn", sb_tn),
                                 ("gh", sb_dbg_gh), ("gi", sb_dbg_gi),
                                 ("w1", sb_dbg_w1)]:
                    eng.dma_start(d_dbg[nm][:, :], sbuf[:, :]).then_inc(s_out, 16)
                    n_out += 1
            eng.wait_ge(s_out, 16 * n_out)

    return nc


def _host_prepack(inputs):
    """All transposes / weight folds in numpy.  Returns (shared, per_core)."""
    dx = np.asarray(inputs["decoder_x"], np.float32)      # [B,T,I]
    x = np.asarray(inputs["x"], np.float32)               # [B,T,3]
    h0 = np.asarray(inputs["h0"], np.float32)             # [B,H]
    W1 = np.asarray(inputs["W1"], np.float32)             # [H,H]
    b1 = np.asarray(inputs["b1"], np.float32)
    W2 = np.asarray(inputs["W2"], np.float32)             # [H,I]
    b2 = np.asarray(inputs["b2"], np.float32)
    W3 = np.asarray(inputs["W3"], np.float32)             # [1,H]
    W4 = np.asarray(inputs["W4"], np.float32)             # [H,I+3]
    b4 = np.asarray(inputs["b4"], np.float32)
    W_ih = np.asarray(inputs["W_ih"], np.float32)         # [3H,H]
    W_hh = np.asarray(inputs["W_hh"], np.float32)
    b_ih = np.asarray(inputs["b_ih"], np.float32)
    b_hh = np.asarray(inputs["b_hh"], np.float32)

    assert np.allclose(b_hh[2 * H:], 0.0), "nonzero b_hh n-gate not supported"

    # W4 fold: gi = [c, x] @ (W_ih @ W4).T + (W_ih @ b4 + b_ih)
    Wa = (W_ih.astype(np.float64) @ W4.astype(np.float64)).astype(np.float32)  # [G, I+3]
    Wa_c, Wa_x = Wa[:, :I], Wa[:, I:]
    bias_g = (W_ih @ b4) + b_ih                           # [G]
    bias_g = bias_g + np.concatenate([b_hh[:2 * H], np.zeros(H, np.float32)])

    # gi_x[b,t,g] = x @ Wa_x.T + bias_g
    gix = np.einsum("btj,gj->btg", x, Wa_x) + bias_g      # [B,T,G]

    # w2[b,t,h] = dx @ W2.T + b2 + b1  (b1 folded from the w1 term)
    w2 = np.einsum("bti,hi->bth", dx, W2) + b2 + b1       # [B,T,H]

    # ---- shared packs ----
    # W1T_pack[p, (kj*4+mj)*128+m] = W1[mj*128+m, kj*128+p]
    W1r = W1.reshape(NKH, 128, NKH, 128)                  # [mj,m,kj,p]
    W1T_pack = np.ascontiguousarray(
        W1r.transpose(3, 2, 0, 1).reshape(128, NKH * NKH * 128))

    Whr = W_hh.reshape(NMG, 128, NKH, 128)                # [mj,m,kj,p]
    WhhT_pack = np.ascontiguousarray(
        Whr.transpose(3, 2, 0, 1).reshape(128, NKH * NMG * 128))

    Wacr = Wa_c.reshape(NMG, 128, 128)                    # [mj,m,p]
    WacT_pack = np.ascontiguousarray(
        Wacr.transpose(2, 0, 1).reshape(128, NMG * 128))

    W3r = np.ascontiguousarray(np.repeat(
        W3.reshape(NKH, 128).T[:, :, None], 128, axis=2).reshape(128, NKH * 128))
    Wfc = np.asarray(inputs["Wfc"], np.float32)
    Wfcc = np.ascontiguousarray(Wfc.reshape(NKH, 128).T)
    shared = dict(W1T=W1T_pack, WhhT=WhhT_pack, WacT=WacT_pack,
                  W3r=W3r, Wfcc=Wfcc)

    per_core = []
    for c in range(NCORES):
        bs = slice(c * BL, (c + 1) * BL)
        dxc = dx[bs]                                      # [16,48,128]
        dxT = np.ascontiguousarray(dxc.transpose(2, 0, 1).reshape(128, BT))
        w2c = w2[bs]                                      # [16,48,512]
        # w2T[p, j*768 + b*48+tau] = w2c[b,tau,j*128+p]
        w2r = w2c.reshape(BL, T, NKH, 128)                # [b,t,j,p]
        w2T = np.ascontiguousarray(
            w2r.transpose(3, 2, 0, 1).reshape(128, NKH * BT))
        gixc = gix[bs]                                    # [16,48,1536]
        gixr = gixc.reshape(BL, T, NMG, 128)              # [b,t,m,p]
        # gixT[p, m*768 + t*16 + b]
        gixT = np.ascontiguousarray(
            gixr.transpose(3, 2, 1, 0).reshape(128, NMG * T * BL))
        h0c = h0[bs]                                      # [16,512]
        h0r = h0c.reshape(BL, NKH, 128)                   # [b,j,p]
        hT0 = np.ascontiguousarray(h0r.transpose(2, 1, 0).reshape(128, NKH * BL))
        per_core.append(dict(dxT=dxT, w2T=w2T, gixT=gixT, hT0=hT0))
    return shared, per_core


def kernel(**inputs):
    from concourse.bass_utils import run_bass_kernel_spmd

    if "nc" not in _PROGRAM_CACHE:
        _PROGRAM_CACHE["nc"] = _build_program()
    nc = _PROGRAM_CACHE["nc"]

    shared, per_core = _host_prepack(inputs)
    in_maps = [dict(shared, **pc) for pc in per_core]
    res = run_bass_kernel_spmd(nc, in_maps, core_ids=list(range(NCORES)))
    _PROGRAM_CACHE["last_results"] = res

    bfc = np.asarray(inputs["bfc"], np.float32)
    re = np.empty((B, 1), np.float32)
    a_f = np.empty((B, T, 1), np.float32)
    for c in range(NCORES):
        r = res.results[c]
        bs = slice(c * BL, (c + 1) * BL)
        re[bs, 0] = r["out_re"][0] + bfc[0]
        aun = r["out_aun"][0].reshape(BL, T)
        asum = r["out_asum"][0].reshape(BL, 1)
        a_f[bs, :, 0] = aun / asum
    return re, a_f


# revision 19
# speedup vs baseline: 2.5499x; 2.5499x over previous
"""Trainium2 Bass kernel for the attention+GRU decoder (nn_Decoder).

Shapes: B=128, T=48, I=128, H=512.  8 NeuronCores, data-parallel over the
batch (16 batch elements per core); the whole 48-step recurrence runs
on-device per core with no cross-core communication.

Math (per batch element b, per step t, h = h_{t-1}):
  w1   = h @ W1.T + b1                       [H]
  e_t  = tanh(w1 + w2_bt) @ W3 + b3          [T]    (w2 precomputed, b3 drops in softmax)
  a    = softmax(e)                          [T]
  c    = sum_t a_t * dx_bt                   [I]
  gi   = [c, x_t] @ (W_ih @ W4).T + (W_ih@b4 + b_ih)   (W4 folded into W_ih => y0 eliminated)
  gh   = h @ W_hh.T + b_hh
  r,z  = sigmoid(...); n = tanh(in + r*hn); h' = (1-z)n + z h
  sigmoid(x) = 0.5 + 0.5*tanh(x/2)  (keeps ACT in the exp_and_others table set)

Device layouts are feature-major ("vertical"): partitions = feature dim,
free dim = batch (16) or (batch, tau).  All weight packing / transposes /
bias folds happen on the host in numpy.
"""

import numpy as np

B, T, I, H = 128, 48, 128, 512
NCORES = 8
BL = B // NCORES  # 16 batch elements per core
BT = BL * T       # 768
G = 3 * H         # 1536 gate dim
NKH = H // 128    # 4   h'-tiles
NMG = G // 128    # 12  gate m-tiles

_PROGRAM_CACHE = {}
MM_BF16 = True
DEBUG_DUMP = False
N_STEPS = T  # recurrence steps (debug knob; attention window stays T)


def _build_program():
    """Build the raw-bass program (one NeuronCore, SPMD across 8)."""
    import concourse.bass as bass
    import concourse.mybir as mybir
    from contextlib import ExitStack

    f32 = mybir.dt.float32
    bf16 = mybir.dt.bfloat16
    mdt = bf16 if MM_BF16 else f32
    Tanh = mybir.ActivationFunctionType.Tanh
    Exp = mybir.ActivationFunctionType.Exp
    Copy = mybir.ActivationFunctionType.Copy
    add = mybir.AluOpType.add
    sub = mybir.AluOpType.subtract
    mult = mybir.AluOpType.mult
    AX = mybir.AxisListType.X

    # Raw-bass kernel: same-engine ordering is guaranteed by the in-order
    # engines; the sim's race detector doesn't model that, so disable it.
    nc = bass.Bass(detect_race_conditions=False)

    # ---- DRAM I/O (per-core tensors) ----
    d_dxT = nc.dram_tensor("dxT", [128, BT], f32, kind="ExternalInput")
    d_w2T = nc.dram_tensor("w2T", [128, NKH * BT], f32, kind="ExternalInput")
    d_gixT = nc.dram_tensor("gixT", [128, NMG * T * BL], f32, kind="ExternalInput")
    d_hT0 = nc.dram_tensor("hT0", [128, NKH * BL], f32, kind="ExternalInput")
    d_hT0m = nc.dram_tensor("hT0m", [128, NKH * BL], mdt, kind="ExternalInput")
    d_W1T = nc.dram_tensor("W1T", [128, NKH * NKH * 128], mdt, kind="ExternalInput")
    d_WhhT = nc.dram_tensor("WhhT", [128, NKH * NMG * 128], mdt, kind="ExternalInput")
    d_WacT = nc.dram_tensor("WacT", [128, NMG * 128], mdt, kind="ExternalInput")
    d_W3r = nc.dram_tensor("W3r", [128, NKH * 128], mdt, kind="ExternalInput")
    d_Wfc = nc.dram_tensor("Wfcc", [128, NKH], f32, kind="ExternalInput")

    d_re = nc.dram_tensor("out_re", [1, BL], f32, kind="ExternalOutput")
    d_dbg = {}
    if DEBUG_DUMP:
        for nm, w in [("cn", BL), ("cu", BL), ("sum", BL), ("rcp", BL),
                      ("adx", BT), ("arep", BT),
                      ("prerz", 8 * BL), ("trz", 8 * BL),
                      ("inn", NKH * BL), ("trhn", NKH * BL), ("q", NKH * BL),
                      ("npre", NKH * BL), ("tn", NKH * BL), ("gh", NMG * BL),
                      ("gi", NMG * BL), ("w1", NKH * BL)]:
            d_dbg[nm] = nc.dram_tensor("dbg_" + nm, [128, w], f32,
                                       kind="ExternalOutput")
    d_outh = nc.dram_tensor("out_h", [128, NKH * BL], f32, kind="ExternalOutput")
    d_aun = nc.dram_tensor("out_aun", [1, BT], f32, kind="ExternalOutput")
    d_asum = nc.dram_tensor("out_asum", [1, BL], f32, kind="ExternalOutput")

    ctx = ExitStack()
    _n = [0]

    def sb(shape, dt=f32):
        _n[0] += 1
        return ctx.enter_context(nc.sbuf_tensor(f"sbt{_n[0]}", shape, dt))

    def ps(shape):
        _n[0] += 1
        return ctx.enter_context(nc.psum_tensor(f"pst{_n[0]}", shape, f32))

    with ctx:
        # ---- SBUF ----
        sb_dxT = sb([128, BT])
        sb_w2T = sb([128, NKH * BT])
        sb_gixT = sb([128, NMG * T * BL])
        sb_hT = sb([128, NKH * BL])          # recurrent state hT (fp32)
        sb_hTm = sb([128, NKH * BL], mdt)    # matmul-rhs copy of hT
        sb_W1T = sb([128, NKH * NKH * 128], mdt)
        sb_WhhT = sb([128, NKH * NMG * 128], mdt)
        sb_WacT = sb([128, NMG * 128], mdt)
        sb_W3r = sb([128, NKH * 128], mdt)
        sb_Wfc = sb([128, NKH])

        sb_z = sb([128, NKH * BT])           # w1 (+) w2, then its tanh
        sb_tanh = sb([128, NKH * BT], mdt)
        sb_arep = sb([128, BT])              # exp(e) un-normalized, replicated
        sb_adx = sb([128, BT])               # a * dxT
        sb_cu = sb([128, BL])                # unnormalized context (vertical)
        sb_sum = sb([128, BL])               # sum_tau a (replicated rows)
        sb_rcp = sb([128, BL])
        sb_cn = sb([128, BL], mdt)           # normalized context
        sb_prerz = sb([128, 8 * BL])
        sb_tmprz = sb([128, 8 * BL])
        sb_trz = sb([128, 8 * BL])
        sb_inn = sb([128, NKH * BL])
        sb_trhn = sb([128, NKH * BL])
        sb_q = sb([128, NKH * BL])
        sb_npre = sb([128, NKH * BL])
        sb_tn = sb([128, NKH * BL])
        sb_p = sb([128, NKH * BL])
        sb_m = sb([128, NKH * BL])
        sb_d2 = sb([128, NKH * BL])
        sb_u = sb([128, NKH * BL])
        sb_re = sb([1, BL])
        sb_dbg_gh = sb([128, NMG * BL]) if DEBUG_DUMP else None
        sb_dbg_gi = sb([128, NMG * BL]) if DEBUG_DUMP else None
        sb_dbg_w1 = sb([128, NKH * BL]) if DEBUG_DUMP else None

        # ---- PSUM (8 banks total) ----
        ps_w1 = ps([128, NKH * BL])          # w1 vertical [h'-tile, b]
        ps_g = ps([128, NMG * BL])           # gh (all 12 m-tiles)
        ps_gi = ps([128, NMG * BL])          # gi (all 12 m-tiles)
        ps_e = ps([128, BT])                 # energies, replicated over partitions
        ps_fc = ps([1, BL])

        s_pe = ctx.enter_context(nc.semaphore("s_pe"))
        s_act = ctx.enter_context(nc.semaphore("s_act"))
        s_dve = ctx.enter_context(nc.semaphore("s_dve"))
        s_dma = ctx.enter_context(nc.semaphore("s_dma"))
        s_out = ctx.enter_context(nc.semaphore("s_out"))
        s_f = ctx.enter_context(nc.semaphore("s_f"))

        block = ctx.enter_context(nc.Block())

        n_in_dmas = 10

        @block.gpsimd
        def _(eng):
            for dst, src in [
                (sb_hT, d_hT0), (sb_hTm, d_hT0m), (sb_W1T, d_W1T), (sb_WhhT, d_WhhT),
                (sb_WacT, d_WacT), (sb_W3r, d_W3r), (sb_Wfc, d_Wfc),
                (sb_dxT, d_dxT), (sb_w2T, d_w2T), (sb_gixT, d_gixT),
            ]:
                eng.dma_start(dst[:, :], src[:, :]).then_inc(s_dma, 16)

        # ------------- PE -------------
        @block.tensor
        def _(eng):
            mm = nc.tensor.matmul
            eng.wait_ge(s_dma, 16 * n_in_dmas)
            for t in range(N_STEPS):
                # P1: w1 = h @ W1.T   (vertical out [h', b])
                if t > 0:
                    eng.wait_ge(s_dve, 5 * (t - 1) + 5)
                last = None
                for mj in range(NKH):
                    for kj in range(NKH):
                        last = mm(ps_w1[:, mj * BL:(mj + 1) * BL],
                                  sb_W1T[:, (kj * NKH + mj) * 128:(kj * NKH + mj + 1) * 128],
                                  sb_hTm[:, kj * BL:(kj + 1) * BL],
                                  start=(kj == 0), stop=(kj == NKH - 1))
                last.then_inc(s_pe, 1)  # 3t+1

                # P2: gh = W_hh @ h  -> ps_g (all 12 m-tiles)
                for mj in range(NMG):
                    for kj in range(NKH):
                        mm(ps_g[:, mj * BL:(mj + 1) * BL],
                           sb_WhhT[:, (kj * NMG + mj) * 128:(kj * NMG + mj + 1) * 128],
                           sb_hTm[:, kj * BL:(kj + 1) * BL],
                           start=(kj == 0), stop=(kj == NKH - 1))

                # P3: e = W3 . tanh(z), output replicated over all partitions
                eng.wait_ge(s_act, 4 * t + 1)
                for kj in range(NKH):
                    mm(ps_e[:, 0:512], sb_W3r[:, kj * 128:(kj + 1) * 128],
                       sb_tanh[:, kj * BT:kj * BT + 512],
                       start=(kj == 0), stop=(kj == NKH - 1))
                last = None
                for kj in range(NKH):
                    last = mm(ps_e[:, 512:BT], sb_W3r[:, kj * 128:(kj + 1) * 128],
                              sb_tanh[:, kj * BT + 512:(kj + 1) * BT],
                              start=(kj == 0), stop=(kj == NKH - 1))
                last.then_inc(s_pe, 1)  # 3t+2

                # P5: gi = Wa_c @ cT (all 12 m-tiles, k = one 128 tile)
                eng.wait_ge(s_dve, 5 * t + 2)
                last = None
                for mj in range(NMG):
                    last = mm(ps_gi[:, mj * BL:(mj + 1) * BL],
                              sb_WacT[:, mj * 128:(mj + 1) * 128], sb_cn[:, :],
                              start=True, stop=True)
                last.then_inc(s_pe, 1)  # 3t+3

            # final: re = Wfc . h_final
            eng.wait_ge(s_dve, 5 * (N_STEPS - 1) + 5)
            last = None
            for kj in range(NKH):
                last = mm(ps_fc[0:1, :], sb_Wfc[:, kj:kj + 1],
                          sb_hT[:, kj * BL:(kj + 1) * BL],
                          start=(kj == 0), stop=(kj == NKH - 1), skip_group_check=True)
            last.then_inc(s_pe, 1)  # 4T+1

        # ------------- ACT -------------
        @block.scalar
        def _(eng):
            act = nc.scalar.activation
            for t in range(N_STEPS):
                eng.wait_ge(s_dve, 5 * t + 1)
                act(sb_tanh[:, :], sb_z[:, :], Tanh).then_inc(s_act, 1)  # 4t+1
                eng.wait_ge(s_pe, 3 * t + 2)
                act(sb_arep[:, :], ps_e[:, :], Exp).then_inc(s_act, 1)  # 4t+2
                eng.wait_ge(s_dve, 5 * t + 3)
                act(sb_trz[:, :], sb_prerz[:, :], Tanh, scale=0.5).then_inc(s_act, 1)  # 4t+3
                eng.wait_ge(s_dve, 5 * t + 4)
                act(sb_tn[:, :], sb_npre[:, :], Tanh).then_inc(s_act, 1)  # 4t+4
            # evac final fc psum
            eng.wait_ge(s_pe, 3 * N_STEPS + 1)
            act(sb_re[0:1, :], ps_fc[0:1, :], Copy).then_inc(s_act, 1)

        # ------------- DVE -------------
        @block.vector
        def _(eng):
            v = nc.vector
            z_out = sb_z[:, :].rearrange("p (j b t) -> p j b t", j=NKH, b=BL)
            w2_in = sb_w2T[:, :].rearrange("p (j b t) -> p j b t", j=NKH, b=BL)
            w1_in = ps_w1[:, :].rearrange("p (j b) -> p j b", j=NKH) \
                               .unsqueeze(3).broadcast_to([128, NKH, BL, T])
            gix_rz = sb_gixT[:, :].rearrange("p (m t b) -> p m t b", m=NMG, t=T)
            adx_v = sb_adx[:, :].rearrange("p (b t) -> p b t", b=BL)
            ar_v = sb_arep[:, :].rearrange("p (b t) -> p b t", b=BL)

            for t in range(N_STEPS):
                # z = w2T + w1 (broadcast over tau)
                eng.wait_ge(s_pe, 3 * t + 1)
                v.tensor_tensor(z_out, w2_in, w1_in, add).then_inc(s_dve, 1)  # 5t+1

                # context: a*dx, reductions, normalize
                eng.wait_ge(s_act, 4 * t + 2)
                v.tensor_tensor(sb_adx[:, :], sb_dxT[:, :], sb_arep[:, :], mult)
                v.tensor_reduce(sb_sum[:, :], ar_v, AX, add)
                v.tensor_reduce(sb_cu[:, :], adx_v, AX, add).then_inc(s_f, 1)
                # reciprocal races adjacent DVE ops on HW: fence before + after
                eng.wait_ge(s_f, 2 * t + 1)
                v.reciprocal(sb_rcp[:, :], sb_sum[:, :]).then_inc(s_f, 1)
                eng.wait_ge(s_f, 2 * t + 2)
                v.tensor_tensor(sb_cn[:, :], sb_cu[:, :], sb_rcp[:, :], mult) \
                 .then_inc(s_dve, 1)  # 5t+2

                # gates
                eng.wait_ge(s_pe, 3 * t + 3)
                v.tensor_tensor(
                    sb_tmprz[:, :].rearrange("p (m b) -> p m b", m=8),
                    ps_g[:, 0:8 * BL].rearrange("p (m b) -> p m b", m=8),
                    gix_rz[:, 0:8, t, :], add)
                v.tensor_tensor(sb_prerz[:, :], sb_tmprz[:, :],
                                ps_gi[:, 0:8 * BL], add).then_inc(s_dve, 1)  # 5t+3
                v.tensor_tensor(
                    sb_inn[:, :].rearrange("p (m b) -> p m b", m=NKH),
                    ps_gi[:, 8 * BL:NMG * BL].rearrange("p (m b) -> p m b", m=NKH),
                    gix_rz[:, 8:12, t, :], add)
                eng.wait_ge(s_act, 4 * t + 3)
                v.tensor_tensor(sb_trhn[:, :], sb_trz[:, 0:NKH * BL],
                                ps_g[:, 8 * BL:NMG * BL], mult)
                v.scalar_tensor_tensor(sb_q[:, :], ps_g[:, 8 * BL:NMG * BL], 0.5,
                                       sb_inn[:, :], mult, add)
                v.scalar_tensor_tensor(sb_npre[:, :], sb_trhn[:, :], 0.5,
                                       sb_q[:, :], mult, add).then_inc(s_dve, 1)  # 5t+4
                eng.wait_ge(s_act, 4 * t + 4)
                v.tensor_scalar_mul(sb_p[:, :], sb_hT[:, :], 0.5)
                v.scalar_tensor_tensor(sb_m[:, :], sb_tn[:, :], 0.5,
                                       sb_p[:, :], mult, add)
                v.scalar_tensor_tensor(sb_d2[:, :], sb_tn[:, :], 0.5,
                                       sb_p[:, :], mult, sub)
                v.tensor_tensor(sb_u[:, :], sb_trz[:, NKH * BL:8 * BL], sb_d2[:, :], mult)
                if DEBUG_DUMP and t == N_STEPS - 1:
                    v.tensor_copy(sb_dbg_gh[:, :], ps_g[:, :])
                    v.tensor_copy(sb_dbg_gi[:, :], ps_gi[:, :])
                    v.tensor_copy(sb_dbg_w1[:, :], ps_w1[:, :])
                v.tensor_tensor(sb_hT[:, :], sb_m[:, :], sb_u[:, :], sub)
                v.tensor_copy(sb_hTm[:, :], sb_hT[:, :]) \
                 .then_inc(s_dve, 1)  # 5t+5

        # ------------- SYNC: outputs -------------
        @block.sync
        def _(eng):
            eng.wait_ge(s_act, 4 * N_STEPS + 1)  # fc evac done
            eng.dma_start(d_re[:, :], sb_re[0:1, :]).then_inc(s_out, 16)
            eng.wait_ge(s_dve, 5 * N_STEPS)
            eng.dma_start(d_aun[:, :], sb_arep[0:1, :]).then_inc(s_out, 16)
            eng.dma_start(d_asum[:, :], sb_sum[0:1, :]).then_inc(s_out, 16)
            eng.dma_start(d_outh[:, :], sb_hT[:, :]).then_inc(s_out, 16)
            n_out = 4
            if DEBUG_DUMP:
                for nm, sbuf in [("cn", sb_cn), ("cu", sb_cu), ("sum", sb_sum),
                                 ("rcp", sb_rcp), ("adx", sb_adx), ("arep", sb_arep),
                                 ("prerz", sb_prerz),
                                 ("trz", sb_trz), ("inn", sb_inn),
                                 ("trhn", sb_trhn), ("q", sb_q),
                                 ("npre", sb_npre), ("tn", sb_tn),
                                 ("gh", sb_dbg_gh), ("gi", sb_dbg_gi),
                                 ("w1", sb_dbg_w1)]:
                    eng.dma_start(d_dbg[nm][:, :], sbuf[:, :]).then_inc(s_out, 16)
                    n_out += 1
            eng.wait_ge(s_out, 16 * n_out)

    return nc


def _host_prepack(inputs):
    """All transposes / weight folds in numpy.  Returns (shared, per_core)."""
    import ml_dtypes
    mnp = ml_dtypes.bfloat16 if MM_BF16 else np.float32
    dx = np.asarray(inputs["decoder_x"], np.float32)      # [B,T,I]
    x = np.asarray(inputs["x"], np.float32)               # [B,T,3]
    h0 = np.asarray(inputs["h0"], np.float32)             # [B,H]
    W1 = np.asarray(inputs["W1"], np.float32)             # [H,H]
    b1 = np.asarray(inputs["b1"], np.float32)
    W2 = np.asarray(inputs["W2"], np.float32)             # [H,I]
    b2 = np.asarray(inputs["b2"], np.float32)
    W3 = np.asarray(inputs["W3"], np.float32)             # [1,H]
    W4 = np.asarray(inputs["W4"], np.float32)             # [H,I+3]
    b4 = np.asarray(inputs["b4"], np.float32)
    W_ih = np.asarray(inputs["W_ih"], np.float32)         # [3H,H]
    W_hh = np.asarray(inputs["W_hh"], np.float32)
    b_ih = np.asarray(inputs["b_ih"], np.float32)
    b_hh = np.asarray(inputs["b_hh"], np.float32)

    assert np.allclose(b_hh[2 * H:], 0.0), "nonzero b_hh n-gate not supported"

    # W4 fold: gi = [c, x] @ (W_ih @ W4).T + (W_ih @ b4 + b_ih)
    Wa = (W_ih.astype(np.float64) @ W4.astype(np.float64)).astype(np.float32)  # [G, I+3]
    Wa_c, Wa_x = Wa[:, :I], Wa[:, I:]
    bias_g = (W_ih @ b4) + b_ih                           # [G]
    bias_g = bias_g + np.concatenate([b_hh[:2 * H], np.zeros(H, np.float32)])

    # gi_x[b,t,g] = x @ Wa_x.T + bias_g
    gix = np.einsum("btj,gj->btg", x, Wa_x) + bias_g      # [B,T,G]

    # w2[b,t,h] = dx @ W2.T + b2 + b1  (b1 folded from the w1 term)
    w2 = np.einsum("bti,hi->bth", dx, W2) + b2 + b1       # [B,T,H]

    # ---- shared packs ----
    # W1T_pack[p, (kj*4+mj)*128+m] = W1[mj*128+m, kj*128+p]
    W1r = W1.reshape(NKH, 128, NKH, 128)                  # [mj,m,kj,p]
    W1T_pack = np.ascontiguousarray(
        W1r.transpose(3, 2, 0, 1).reshape(128, NKH * NKH * 128))

    Whr = W_hh.reshape(NMG, 128, NKH, 128)                # [mj,m,kj,p]
    WhhT_pack = np.ascontiguousarray(
        Whr.transpose(3, 2, 0, 1).reshape(128, NKH * NMG * 128))

    Wacr = Wa_c.reshape(NMG, 128, 128)                    # [mj,m,p]
    WacT_pack = np.ascontiguousarray(
        Wacr.transpose(2, 0, 1).reshape(128, NMG * 128))

    W3r = np.ascontiguousarray(np.repeat(
        W3.reshape(NKH, 128).T[:, :, None], 128, axis=2).reshape(128, NKH * 128))
    Wfc = np.asarray(inputs["Wfc"], np.float32)
    Wfcc = np.ascontiguousarray(Wfc.reshape(NKH, 128).T)
    shared = dict(W1T=W1T_pack.astype(mnp), WhhT=WhhT_pack.astype(mnp),
                  WacT=WacT_pack.astype(mnp), W3r=W3r.astype(mnp), Wfcc=Wfcc)

    per_core = []
    for c in range(NCORES):
        bs = slice(c * BL, (c + 1) * BL)
        dxc = dx[bs]                                      # [16,48,128]
        dxT = np.ascontiguousarray(dxc.transpose(2, 0, 1).reshape(128, BT))
        w2c = w2[bs]                                      # [16,48,512]
        # w2T[p, j*768 + b*48+tau] = w2c[b,tau,j*128+p]
        w2r = w2c.reshape(BL, T, NKH, 128)                # [b,t,j,p]
        w2T = np.ascontiguousarray(
            w2r.transpose(3, 2, 0, 1).reshape(128, NKH * BT))
        gixc = gix[bs]                                    # [16,48,1536]
        gixr = gixc.reshape(BL, T, NMG, 128)              # [b,t,m,p]
        # gixT[p, m*768 + t*16 + b]
        gixT = np.ascontiguousarray(
            gixr.transpose(3, 2, 1, 0).reshape(128, NMG * T * BL))
        h0c = h0[bs]                                      # [16,512]
        h0r = h0c.reshape(BL, NKH, 128)                   # [b,j,p]
        hT0 = np.ascontiguousarray(h0r.transpose(2, 1, 0).reshape(128, NKH * BL))
        per_core.append(dict(dxT=dxT, w2T=w2T, gixT=gixT, hT0=hT0,
                             hT0m=hT0.astype(mnp)))
    return shared, per_core


def kernel(**inputs):
    from concourse.bass_utils import run_bass_kernel_spmd

    if "nc" not in _PROGRAM_CACHE:
        _PROGRAM_CACHE["nc"] = _build_program()
    nc = _PROGRAM_CACHE["nc"]

    shared, per_core = _host_prepack(inputs)
    in_maps = [dict(shared, **pc) for pc in per_core]
    res = run_bass_kernel_spmd(nc, in_maps, core_ids=list(range(NCORES)))
    _PROGRAM_CACHE["last_results"] = res

    bfc = np.asarray(inputs["bfc"], np.float32)
    re = np.empty((B, 1), np.float32)
    a_f = np.empty((B, T, 1), np.float32)
    for c in range(NCORES):
        r = res.results[c]
        bs = slice(c * BL, (c + 1) * BL)
        re[bs, 0] = r["out_re"][0] + bfc[0]
        aun = r["out_aun"][0].reshape(BL, T)
        asum = r["out_asum"][0].reshape(BL, 1)
        a_f[bs, :, 0] = aun / asum
    return re, a_f
